# revision 24
# baseline (speedup 1.0000x reference)
"""Distance-weighted self-attention on 8 Trainium2 NeuronCores.

Data-parallel over batch: B=8 batches -> 1 batch element per core, no
collectives.  Per core (N=2048 tokens, D=128):

  q = x Wq / sqrt(D), k = x Wk, v = x Wv
  l[i,j] = (q_i . k_j) * exp(-lambda |a_i - a_j|)
  out = softmax_j(l) V Wo

Tokens are SORTED by allele size on the host (attention is
permutation-equivariant).  After sorting, for a key strip covering
sorted positions [128k, 128k+128) the decay factorizes around the
diagonal:
  exp(-l|a_m - a_p|) = (e^{-l a_m} e^{+l a_p})   for a_m >= a_p
                     = (e^{+l a_m} e^{-l a_p})   for a_m <= a_p
The host pre-scales x into xm = x*e^{-l a} and xp = x*e^{+l a}; the
decayed scores come straight out of the Q/K matmuls.  Only the 16
diagonal 128x128 blocks need a fix-up multiply by
exp(2*lambda*min(a_m - a_p, 0)).

Pipeline (all matmul inputs bf16; PSUM accumulation fp32):
  - chunked DMA loads so the first score matmul runs ~3us in (keeps
    the PE HAM clock ramped; dummy warmup matmuls cover the start)
  - projections qm/qp/km/kp/v: bf16 matmuls, DVE evacuates PSUM->bf16
  - per key strip: scores into [128,1024] PSUM tiles (x2 buffers,
    2 banks each; ctx holds the other 4 banks), diagonal fix-up on
    DVE, ONE wide exp per tile on ACT writing p as bf16 (bias carries
    ln(mask))
  - ctx^T accumulates over strips in PSUM via bf16 matmuls
  - softmax sums: DVE running bf16 accumulator over p (keeps the
    ~14us of ones-matmuls off the busy tensor engine), finished by
    four ones-matmuls into PSUM banks 4-7 after the last strip
  - epilogue per 512-query chunk: 1/sums = exp(-ln(sums)), PE
    broadcast to 128 partitions, DVE scales ctx, Wo matmul, DMA out
    (chunks pipeline; banks 4-7 recycled sums->bc/out)
"""

import numpy as np

B, N, D = 8, 2048, 128
PB = 128            # keys per strip (partition block)
LAMBDA_DECAY = 0.1

_CACHE = {}


def _split_drain_waits(bir: bytes, limit: int = 1) -> bytes:
    """This container's walrus rejects instructions carrying more than
    `limit` sync waits ("Too many sync wait commands", setupSyncWait).
    Tile freely attaches several waits to one instruction.  For any
    over-limit instruction, hoist the overflow waits onto same-engine
    EventSemaphore instructions inserted immediately before it
    (same-engine program order preserves the semantics)."""
    import json

    m = json.loads(bir)

    def fix(obj):
        if isinstance(obj, dict):
            if "instructions" in obj and isinstance(obj["instructions"], list):
                out = []
                for ins in obj["instructions"]:
                    si = ins.get("sync_info")
                    if si and si.get("on_wait") and len(si["on_wait"]) > limit:
                        waits = si["on_wait"]
                        chunks = [
                            waits[i:i + limit]
                            for i in range(0, len(waits), limit)
                        ]
                        for j, ch in enumerate(chunks[:-1]):
                            out.append({
                                "name": f"{ins['name']}_w{j}",
                                "opcode": "EventSemaphore",
                                "engine": ins["engine"],
                                "debug": ins.get("debug", 0),
                                "ins": [],
                                "outs": [],
                                "sync_info": {"on_update": [], "on_wait": ch},
                            })
                        si["on_wait"] = chunks[-1]
                    out.append(ins)
                obj["instructions"] = out
            for v in obj.values():
                fix(v)
        elif isinstance(obj, list):
            for v in obj:
                fix(v)

    fix(m)
    return json.dumps(m).encode()


def _build(n=N):
    from contextlib import ExitStack

    import concourse.bass as bass
    import concourse.tile as tile
    from concourse import mybir

    f32 = mybir.dt.float32
    f32r = mybir.dt.float32r
    bf16 = mybir.dt.bfloat16
    Act = mybir.ActivationFunctionType
    Alu = mybir.AluOpType

    nkb = n // PB          # 16 key strips
    nqc = n // 512         # 4 query chunks
    HW = 1024              # exp tile width (2 PSUM banks)
    nh = n // HW           # halves per strip

    f16 = mybir.dt.float16

    nc = bass.Bass("TRN2", target_bir_lowering=False, debug=False)
    xT_d = nc.declare_dram_parameter("xT", [D, n], bf16, isOutput=False)
    xmT_d = nc.declare_dram_parameter("xmT", [D, n], bf16, isOutput=False)
    xpT_d = nc.declare_dram_parameter("xpT", [D, n], bf16, isOutput=False)
    ra_d = nc.declare_dram_parameter("ra", [128, n], f16, isOutput=False)
    ak_d = nc.declare_dram_parameter("ak", [128, nkb], f32, isOutput=False)
    lnm_d = nc.declare_dram_parameter("lnm", [128, nkb], f32, isOutput=False)
    wq_d = nc.declare_dram_parameter("wq", [D, D], bf16, isOutput=False)
    wk_d = nc.declare_dram_parameter("wk", [D, D], bf16, isOutput=False)
    wv_d = nc.declare_dram_parameter("wv", [D, D], bf16, isOutput=False)
    wo_d = nc.declare_dram_parameter("wo", [D, D], bf16, isOutput=False)
    outT_d = nc.declare_dram_parameter("outT", [D, n], bf16, isOutput=True)

    with tile.TileContext(nc) as tc:
        with ExitStack() as ctx:
            const = ctx.enter_context(tc.tile_pool(name="const", bufs=1))

            # ---- loads: one ring per engine, earliest-needed first -----
            # sync/scalar are HWDGE rings, gpsimd software ring; spread
            # the big tensors and order by first use
            wq = const.tile([D, D], bf16, tag="wq")
            wk = const.tile([D, D], bf16, tag="wk")
            wv = const.tile([D, D], bf16, tag="wv")
            wo = const.tile([D, D], bf16, tag="wo")
            ak = const.tile([128, nkb], f32, tag="ak")
            lnm = const.tile([128, nkb], f32, tag="lnm")
            ra = const.tile([128, n], f16, tag="ra")
            xT = const.tile([D, n], bf16, tag="xT")
            xmT = const.tile([D, n], bf16, tag="xmT")
            xpT = const.tile([D, n], bf16, tag="xpT")

            nc.gpsimd.dma_start(ra[:, 0:256], ra_d[:, 0:256])
            nc.gpsimd.dma_start(wq[:], wq_d[:])
            nc.gpsimd.dma_start(wk[:], wk_d[:])
            nc.gpsimd.dma_start(ak[:], ak_d[:])
            nc.gpsimd.dma_start(lnm[:], lnm_d[:])
            h2 = n // 2
            for c in range(2):
                sl = slice(c * h2, (c + 1) * h2)
                nc.sync.dma_start(xmT[:, sl], xmT_d[:, sl])
                nc.scalar.dma_start(xpT[:, sl], xpT_d[:, sl])
                nc.gpsimd.dma_start(xT[:, sl], xT_d[:, sl])
            nc.gpsimd.dma_start(wv[:], wv_d[:])
            nc.gpsimd.dma_start(wo[:], wo_d[:])
            # band inputs for later strips ride the HWDGE ring tails
            nc.sync.dma_start(ra[:, 256:1152], ra_d[:, 256:1152])
            nc.scalar.dma_start(ra[:, 1152:n], ra_d[:, 1152:n])

            ones_bf = const.tile([128, 1], bf16)
            nc.vector.memset(ones_bf[:], 1.0)
            ones_f = const.tile([1, 128], f32)
            nc.vector.memset(ones_f[:], 1.0)
            ones_r = const.tile([1, 128], f32r)
            nc.vector.tensor_copy(ones_r[:], ones_f[:])

            # ---- projections (bf16), PE warmup -------------------------
            qmT = const.tile([D, n], bf16)   # q from xm  (right region)
            qpT = const.tile([D, n], bf16)   # q from xp  (left region)
            kmT = const.tile([D, n], bf16)   # k from xm  (left stationary)
            kpT = const.tile([D, n], bf16)   # k from xp  (right stationary)
            v_sb = const.tile([128, n], bf16)  # strip k at cols [128k,...)

            # PE HAM warmup: dummy matmuls with no DMA deps flip the PE
            # clock gate to 8/8 during the load window
            warm_w = const.tile([128, 128], f32)
            warm_x = const.tile([128, 512], f32)
            nc.vector.memset(warm_w[:], 0.5)
            nc.vector.memset(warm_x[:], 0.5)

            # diag-band precompute buffer (in-place exp over it later)
            bands = const.tile([128, n], f32)

            # diag-band prep on GPSIMD (idle engine, SBUF only):
            # bands = min(a_m - a_p, 0) per strip; exp for strips 0-1
            # here, the rest deferred into the loop (ra tail DMAs land
            # late and must not block the scalar queue)
            for k in range(nkb):
                lo = k * PB
                nc.gpsimd.tensor_scalar(
                    bands[:, lo:lo + PB], ra[:, lo:lo + PB],
                    ak[:, k:k + 1], 0.0, Alu.subtract, Alu.min)
            nc.scalar.activation(
                bands[:, 0:256], bands[:, 0:256],
                Act.Exp, scale=2.0 * LAMBDA_DECAY)

            # projections, need-ordered: strip 0 scores need kp strip 0
            # and ALL qm chunks; km/qp follow for later strips; v last.
            # Evacuations split ACT (first few, unblocks strip 0) / DVE.
            with tc.tile_pool(name="proj_ps", bufs=3, space="PSUM") as proj_ps:
                wt = proj_ps.tile([128, 512], f32, tag="warm")
                for i in range(6):
                    nc.tensor.matmul(
                        wt, warm_w[:], warm_x[:],
                        start=(i == 0), stop=(i == 5))
                jobs = []
                for c in range(4):
                    jobs.append((kpT, wk, xpT, c))
                    jobs.append((qmT, wq, xmT, c))
                for c in range(4):
                    jobs.append((kmT, wk, xmT, c))
                    jobs.append((qpT, wq, xpT, c))
                act_evac = {(id(kpT), 0), (id(qmT), 0), (id(qmT), 1)}
                for dst, w, src, c in jobs:
                    sl = slice(c * 512, (c + 1) * 512)
                    t = proj_ps.tile([D, 512], f32, tag="proj")
                    nc.tensor.matmul(
                        t, w[:], src[:, sl], start=True, stop=True)
                    if (id(dst), c) in act_evac:
                        nc.scalar.activation(dst[:, sl], t, Act.Copy)
                    else:
                        nc.vector.tensor_copy(dst[:, sl], t)
                # v strips: v_blk = x_blk @ Wv (keys on partitions)
                for k4 in range(0, nkb, 4):
                    t = proj_ps.tile([128, 4 * PB], f32, tag="proj")
                    for k in range(k4, k4 + 4):
                        nc.tensor.matmul(
                            t[:, (k - k4) * PB:(k - k4 + 1) * PB],
                            xT[:, k * PB:(k + 1) * PB], wv[:],
                            start=True, stop=True)
                    nc.vector.tensor_copy(
                        v_sb[:, k4 * PB:(k4 + 4) * PB], t)

            # ---- main loop over key strips ------------------------------
            acc_ps = ctx.enter_context(
                tc.tile_pool(name="acc_ps", bufs=1, space="PSUM"))
            ctxT_ps = acc_ps.tile([128, n], f32)

            acc_bf = const.tile([128, n], bf16)   # running softmax sums

            with (
                tc.tile_pool(name="s_ps", bufs=2, space="PSUM") as s_ps,
                tc.tile_pool(name="p_sb", bufs=3) as p_pool,
            ):
                p_ts = [None] * nkb

                def consume(k):
                    # ctx^T += v_strip^T @ p_k, and softmax-sum acc
                    lo, hi = k * PB, (k + 1) * PB
                    for c in range(nqc):
                        nc.tensor.matmul(
                            ctxT_ps[:, c * 512:(c + 1) * 512],
                            v_sb[:, lo:hi],
                            p_ts[k][:, c * 512:(c + 1) * 512],
                            start=(k == 0), stop=(k == nkb - 1))
                    if k == 0:
                        nc.vector.tensor_copy(acc_bf[:], p_ts[k][:])
                    else:
                        nc.vector.tensor_add(
                            acc_bf[:], acc_bf[:], p_ts[k][:])

                for k in range(nkb):
                    lo, hi = k * PB, (k + 1) * PB
                    if k == 2:
                        nc.scalar.activation(
                            bands[:, 256:1152], bands[:, 256:1152],
                            Act.Exp, scale=2.0 * LAMBDA_DECAY)
                    elif k == 9:
                        nc.scalar.activation(
                            bands[:, 1152:n], bands[:, 1152:n],
                            Act.Exp, scale=2.0 * LAMBDA_DECAY)
                    p_ts[k] = p_pool.tile(
                        [128, n], bf16, tag="p", name=f"p{k}")
                    s_ts = [s_ps.tile([128, HW], f32, tag="s",
                                      name=f"s{k}_{h}")
                            for h in range(nh)]
                    # left region [0, lo) from (kmT, qpT); right+diag
                    # [lo, n) from (kpT, qmT).  Same-stationary matmuls
                    # back-to-back to amortize LDWEIGHTS.
                    for h in range(nh):
                        c0, c1 = h * HW, (h + 1) * HW
                        for q0 in range(c0, min(c1, lo), 512):
                            e = min(q0 + 512, lo)
                            nc.tensor.matmul(
                                s_ts[h][:, q0 - c0:e - c0], kmT[:, lo:hi],
                                qpT[:, q0:e], start=True, stop=True)
                    for h in range(nh):
                        c0, c1 = h * HW, (h + 1) * HW
                        for q0 in range(c0, c1, 512):
                            q1 = q0 + 512
                            if q1 > lo:
                                b = max(q0, lo)
                                nc.tensor.matmul(
                                    s_ts[h][:, b - c0:q1 - c0],
                                    kpT[:, lo:hi],
                                    qmT[:, b:q1], start=True, stop=True)
                        if c0 <= lo < c1:
                            o = lo - c0
                            nc.vector.tensor_mul(
                                s_ts[h][:, o:o + PB], s_ts[h][:, o:o + PB],
                                bands[:, lo:hi])
                        nc.scalar.activation(
                            p_ts[k][:, c0:c1], s_ts[h][:], Act.Exp,
                            bias=lnm[:, k:k + 1])
                    # software pipeline: consume the PREVIOUS strip's p
                    # so the PE never sits behind ctx waiting on this
                    # strip's exps
                    if k > 0:
                        consume(k - 1)
                consume(nkb - 1)

            # ---- epilogue ----------------------------------------------
            # normalize AFTER Wo (per-query scale commutes through the
            # feature-mixing matmul): out = (Wo^T ctx_raw) * bc(1/sums).
            # ctx evacuation (DVE) overlaps the sums ln/exp chain (ACT);
            # banks 4-7 recycle sums -> bc/out.
            lns = const.tile([1, n], f32)
            invr = const.tile([1, n], f32r)
            ctx_bf = const.tile([128, n], bf16)
            bc_sbs = [const.tile([128, 512], f32, name=f"bc_sb{i}")
                      for i in range(2)]
            outT_sb = const.tile([D, n], bf16)

            for c in range(nqc):
                sl = slice(c * 512, (c + 1) * 512)
                nc.vector.tensor_copy(ctx_bf[:, sl], ctxT_ps[:, sl])

            with tc.tile_pool(name="sums_ps", bufs=1, space="PSUM") as sums_pool:
                sums_t = sums_pool.tile([1, n], f32)
                for c in range(nqc):
                    nc.tensor.matmul(
                        sums_t[0:1, c * 512:(c + 1) * 512], ones_bf[:],
                        acc_bf[:, c * 512:(c + 1) * 512],
                        start=True, stop=True)
                for h in range(2):
                    sl = slice(h * 1024, (h + 1) * 1024)
                    nc.scalar.activation(lns[0:1, sl], sums_t[0:1, sl],
                                         Act.Ln)
                    nc.scalar.activation(invr[0:1, sl], lns[0:1, sl],
                                         Act.Exp, scale=-1.0)

            with (
                tc.tile_pool(name="bc_ps", bufs=2, space="PSUM") as bc_pool,
                tc.tile_pool(name="o_ps", bufs=2, space="PSUM") as o_pool,
            ):
                for c in range(nqc):
                    sl = slice(c * 512, (c + 1) * 512)
                    bc_ps = bc_pool.tile([128, 512], f32, tag="bc")
                    nc.tensor.matmul(
                        bc_ps, ones_r[:], invr[0:1, sl],
                        start=True, stop=True)
                    bc_s = bc_sbs[c % 2]
                    nc.scalar.activation(bc_s[:], bc_ps, Act.Copy)
                    o_t = o_pool.tile([D, 512], f32, tag="o")
                    nc.tensor.matmul(
                        o_t, wo[:], ctx_bf[:, sl], start=True, stop=True)
                    nc.vector.tensor_mul(outT_sb[:, sl], o_t, bc_s[:])
                    eng = nc.sync if c % 2 == 0 else nc.scalar
                    eng.dma_start(outT_d[:, sl], outT_sb[:, sl])

    orig_to_json = nc.to_json_bytes
    nc.to_json_bytes = lambda *a, **kw: _split_drain_waits(orig_to_json(*a, **kw))
    return nc


def _in_maps(inputs, allele_sizes, mask, Wq, Wk, Wv, Wo):
    import ml_dtypes

    bf = ml_dtypes.bfloat16
    n = inputs.shape[1]
    nkb = n // PB
    wq = np.ascontiguousarray(Wq / np.sqrt(np.float32(D))).astype(bf)
    wk = np.ascontiguousarray(Wk).astype(bf)
    wv = np.ascontiguousarray(Wv).astype(bf)
    wo = np.ascontiguousarray(Wo).astype(bf)
    maps = []
    perms = []
    for b in range(inputs.shape[0]):
        a_raw = np.asarray(allele_sizes[b], dtype=np.float64)
        perm = np.argsort(a_raw, kind="stable")
        perms.append(perm)
        a = a_raw[perm]
        a = a - a.mean()          # shift-invariant; shrinks e^{±la} range
        x = np.asarray(inputs[b], dtype=np.float64)[perm]
        m = np.asarray(mask[b], dtype=np.float32)[perm]
        em = np.exp(-LAMBDA_DECAY * a)
        ep = np.exp(LAMBDA_DECAY * a)
        xm = (x * em[:, None]).astype(bf)
        xp = (x * ep[:, None]).astype(bf)
        xb = x.astype(bf)
        a = a.astype(np.float32)
        maps.append({
            "xT": np.ascontiguousarray(xb.T),
            "xmT": np.ascontiguousarray(xm.T),
            "xpT": np.ascontiguousarray(xp.T),
            "ra": np.ascontiguousarray(
                np.broadcast_to(a[None, :], (128, n))).astype(np.float16),
            "ak": np.ascontiguousarray(a.reshape(nkb, PB).T),
            "lnm": np.ascontiguousarray(
                np.log(m.reshape(nkb, PB).T,
                       where=m.reshape(nkb, PB).T > 0,
                       out=np.full((PB, nkb), -np.inf, dtype=np.float32))),
            "wq": wq, "wk": wk, "wv": wv, "wo": wo,
        })
    return maps, perms


LAST_RESULTS = None


def kernel(inputs, allele_sizes, mask, Wq, Wk, Wv, Wo, **run_kwargs):
    global LAST_RESULTS
    from concourse.bass_utils import run_bass_kernel_spmd

    key = ("nc", inputs.shape[1])
    if key not in _CACHE:
        _CACHE[key] = _build(n=inputs.shape[1])
    nc = _CACHE[key]
    maps, perms = _in_maps(inputs, allele_sizes, mask, Wq, Wk, Wv, Wo)
    res = run_bass_kernel_spmd(nc, maps, list(range(len(maps))), **run_kwargs)
    LAST_RESULTS = res
    outs = []
    for b, perm in enumerate(perms):
        o_sorted = np.asarray(
            res.results[b]["outT"].T, dtype=np.float32)  # [n, D], sorted
        o = np.empty_like(o_sorted)
        o[perm] = o_sorted
        outs.append(o)
    return np.stack(outs).astype(np.float32)


# revision 30
# speedup vs baseline: 1.3639x; 1.3639x over previous
"""Distance-weighted self-attention on 8 Trainium2 NeuronCores.

Data-parallel over batch: B=8 batches -> 1 batch element per core, no
collectives.  Per core (N=2048 tokens, D=128):

  q = x Wq / sqrt(D), k = x Wk, v = x Wv
  l[i,j] = (q_i . k_j) * exp(-lambda |a_i - a_j|)
  out = softmax_j(l) V Wo

Tokens are SORTED by allele size on the host (attention is
permutation-equivariant).  After sorting, for a key strip covering
sorted positions [128k, 128k+128) the decay factorizes around the
diagonal:
  exp(-l|a_m - a_p|) = (e^{-l a_m} e^{+l a_p})   for a_m >= a_p
                     = (e^{+l a_m} e^{-l a_p})   for a_m <= a_p
The host pre-scales x into xm = x*e^{-l a} and xp = x*e^{+l a}; the
decayed scores come straight out of the Q/K matmuls.  Only the 16
diagonal 128x128 blocks need a fix-up multiply by
exp(2*lambda*min(a_m - a_p, 0)).

Pipeline (all matmul inputs bf16; PSUM accumulation fp32):
  - chunked DMA loads so the first score matmul runs ~3us in (keeps
    the PE HAM clock ramped; dummy warmup matmuls cover the start)
  - projections qm/qp/km/kp/v: bf16 matmuls, DVE evacuates PSUM->bf16
  - per key strip: scores into [128,1024] PSUM tiles (x2 buffers,
    2 banks each; ctx holds the other 4 banks), diagonal fix-up on
    DVE, ONE wide exp per tile on ACT writing p as bf16 (bias carries
    ln(mask))
  - ctx^T accumulates over strips in PSUM via bf16 matmuls
  - softmax sums: DVE running bf16 accumulator over p (keeps the
    ~14us of ones-matmuls off the busy tensor engine), finished by
    four ones-matmuls into PSUM banks 4-7 after the last strip
  - epilogue per 512-query chunk: 1/sums = exp(-ln(sums)), PE
    broadcast to 128 partitions, DVE scales ctx, Wo matmul, DMA out
    (chunks pipeline; banks 4-7 recycled sums->bc/out)
"""

import numpy as np

B, N, D = 8, 2048, 128
PB = 128            # keys per strip (partition block)
LAMBDA_DECAY = 0.1

_CACHE = {}


def _split_drain_waits(bir: bytes, limit: int = 1) -> bytes:
    """This container's walrus rejects instructions carrying more than
    `limit` sync waits ("Too many sync wait commands", setupSyncWait).
    Tile freely attaches several waits to one instruction.  For any
    over-limit instruction, hoist the overflow waits onto same-engine
    EventSemaphore instructions inserted immediately before it
    (same-engine program order preserves the semantics)."""
    import json

    m = json.loads(bir)

    def fix(obj):
        if isinstance(obj, dict):
            if "instructions" in obj and isinstance(obj["instructions"], list):
                out = []
                for ins in obj["instructions"]:
                    si = ins.get("sync_info")
                    if si and si.get("on_wait") and len(si["on_wait"]) > limit:
                        waits = si["on_wait"]
                        chunks = [
                            waits[i:i + limit]
                            for i in range(0, len(waits), limit)
                        ]
                        for j, ch in enumerate(chunks[:-1]):
                            out.append({
                                "name": f"{ins['name']}_w{j}",
                                "opcode": "EventSemaphore",
                                "engine": ins["engine"],
                                "debug": ins.get("debug", 0),
                                "ins": [],
                                "outs": [],
                                "sync_info": {"on_update": [], "on_wait": ch},
                            })
                        si["on_wait"] = chunks[-1]
                    out.append(ins)
                obj["instructions"] = out
            for v in obj.values():
                fix(v)
        elif isinstance(obj, list):
            for v in obj:
                fix(v)

    fix(m)
    return json.dumps(m).encode()


def _build(n=N):
    from contextlib import ExitStack

    import concourse.bass as bass
    import concourse.tile as tile
    from concourse import mybir

    f32 = mybir.dt.float32
    f32r = mybir.dt.float32r
    bf16 = mybir.dt.bfloat16
    Act = mybir.ActivationFunctionType
    Alu = mybir.AluOpType

    nkb = n // PB          # 16 key strips
    nqc = n // 512         # 4 query chunks
    HW = 1024              # exp tile width (2 PSUM banks)
    nh = n // HW           # halves per strip

    f16 = mybir.dt.float16

    nc = bass.Bass("TRN2", target_bir_lowering=False, debug=False)
    xT_d = nc.declare_dram_parameter("xT", [D, n], bf16, isOutput=False)
    xmT_d = nc.declare_dram_parameter("xmT", [D, n], bf16, isOutput=False)
    xpT_d = nc.declare_dram_parameter("xpT", [D, n], bf16, isOutput=False)
    bands_d = nc.declare_dram_parameter("bands", [128, n], f16, isOutput=False)
    lnm_d = nc.declare_dram_parameter("lnm", [128, nkb], f32, isOutput=False)
    wq_d = nc.declare_dram_parameter("wq", [D, D], bf16, isOutput=False)
    wk_d = nc.declare_dram_parameter("wk", [D, D], bf16, isOutput=False)
    wv_d = nc.declare_dram_parameter("wv", [D, D], bf16, isOutput=False)
    wo_d = nc.declare_dram_parameter("wo", [D, D], bf16, isOutput=False)
    outT_d = nc.declare_dram_parameter("outT", [D, n], bf16, isOutput=True)

    with tile.TileContext(nc) as tc:
        with ExitStack() as ctx:
            const = ctx.enter_context(tc.tile_pool(name="const", bufs=1))

            # ---- loads: one ring per engine, earliest-needed first -----
            # sync/scalar are HWDGE rings, gpsimd software ring; spread
            # the big tensors and order by first use
            wq = const.tile([D, D], bf16, tag="wq")
            wk = const.tile([D, D], bf16, tag="wk")
            wv = const.tile([D, D], bf16, tag="wv")
            wo = const.tile([D, D], bf16, tag="wo")
            lnm = const.tile([128, nkb], f32, tag="lnm")
            bands = const.tile([128, n], f16, tag="bands")
            xT = const.tile([D, n], bf16, tag="xT")
            xmT = const.tile([D, n], bf16, tag="xmT")
            xpT = const.tile([D, n], bf16, tag="xpT")

            nc.gpsimd.dma_start(bands[:, 0:256], bands_d[:, 0:256])
            nc.gpsimd.dma_start(wq[:], wq_d[:])
            nc.gpsimd.dma_start(wk[:], wk_d[:])
            nc.gpsimd.dma_start(lnm[:], lnm_d[:])
            h2 = n // 2
            for c in range(2):
                sl = slice(c * h2, (c + 1) * h2)
                nc.sync.dma_start(xmT[:, sl], xmT_d[:, sl])
                nc.scalar.dma_start(xpT[:, sl], xpT_d[:, sl])
                nc.gpsimd.dma_start(xT[:, sl], xT_d[:, sl])
            nc.gpsimd.dma_start(wv[:], wv_d[:])
            nc.gpsimd.dma_start(wo[:], wo_d[:])
            # band blocks for later strips ride the HWDGE ring tails
            nc.sync.dma_start(bands[:, 256:1152], bands_d[:, 256:1152])
            nc.scalar.dma_start(bands[:, 1152:n], bands_d[:, 1152:n])

            ones_bf = const.tile([128, 1], bf16)
            nc.vector.memset(ones_bf[:], 1.0)
            ones_f = const.tile([1, 128], f32)
            nc.vector.memset(ones_f[:], 1.0)
            ones_r = const.tile([1, 128], f32r)
            nc.vector.tensor_copy(ones_r[:], ones_f[:])

            # ---- projections (bf16), PE warmup -------------------------
            qmT = const.tile([D, n], bf16)   # q from xm  (right region)
            qpT = const.tile([D, n], bf16)   # q from xp  (left region)
            kmT = const.tile([D, n], bf16)   # k from xm  (left stationary)
            kpT = const.tile([D, n], bf16)   # k from xp  (right stationary)
            v_sb = const.tile([128, n], bf16)  # strip k at cols [128k,...)

            # PE HAM warmup: dummy matmuls with no DMA deps flip the PE
            # clock gate to 8/8 during the load window
            warm_w = const.tile([128, 128], f32)
            warm_x = const.tile([128, 512], f32)
            nc.vector.memset(warm_w[:], 0.5)
            nc.vector.memset(warm_x[:], 0.5)


            # projections, need-ordered: strip 0 scores need kp strip 0
            # and ALL qm chunks; km/qp follow for later strips; v last.
            # Evacuations split ACT (first few, unblocks strip 0) / DVE.
            with tc.tile_pool(name="proj_ps", bufs=3, space="PSUM") as proj_ps:
                wt = proj_ps.tile([128, 512], f32, tag="warm")
                for i in range(6):
                    nc.tensor.matmul(
                        wt, warm_w[:], warm_x[:],
                        start=(i == 0), stop=(i == 5))
                jobs = []
                for c in range(4):
                    jobs.append((kpT, wk, xpT, c))
                    jobs.append((qmT, wq, xmT, c))
                for c in range(4):
                    jobs.append((kmT, wk, xmT, c))
                    jobs.append((qpT, wq, xpT, c))
                act_evac = {(id(kpT), 0), (id(qmT), 0), (id(qmT), 1)}
                for dst, w, src, c in jobs:
                    sl = slice(c * 512, (c + 1) * 512)
                    t = proj_ps.tile([D, 512], f32, tag="proj")
                    nc.tensor.matmul(
                        t, w[:], src[:, sl], start=True, stop=True)
                    if (id(dst), c) in act_evac:
                        nc.scalar.activation(dst[:, sl], t, Act.Copy)
                    else:
                        nc.vector.tensor_copy(dst[:, sl], t)
                # v strips: v_blk = x_blk @ Wv (keys on partitions)
                for k4 in range(0, nkb, 4):
                    t = proj_ps.tile([128, 4 * PB], f32, tag="proj")
                    for k in range(k4, k4 + 4):
                        nc.tensor.matmul(
                            t[:, (k - k4) * PB:(k - k4 + 1) * PB],
                            xT[:, k * PB:(k + 1) * PB], wv[:],
                            start=True, stop=True)
                    nc.vector.tensor_copy(
                        v_sb[:, k4 * PB:(k4 + 4) * PB], t)

            # ---- main loop over key strips ------------------------------
            acc_ps = ctx.enter_context(
                tc.tile_pool(name="acc_ps", bufs=1, space="PSUM"))
            ctxT_ps = acc_ps.tile([128, n], f32)

            acc_bf = const.tile([128, n], bf16)   # running softmax sums

            with (
                tc.tile_pool(name="s_ps", bufs=2, space="PSUM") as s_ps,
                tc.tile_pool(name="p_sb", bufs=3) as p_pool,
            ):
                p_ts = [None] * nkb

                def consume(k):
                    # ctx^T += v_strip^T @ p_k, and softmax-sum acc
                    lo, hi = k * PB, (k + 1) * PB
                    for c in range(nqc):
                        nc.tensor.matmul(
                            ctxT_ps[:, c * 512:(c + 1) * 512],
                            v_sb[:, lo:hi],
                            p_ts[k][:, c * 512:(c + 1) * 512],
                            start=(k == 0), stop=(k == nkb - 1))
                    if k == 0:
                        nc.vector.tensor_copy(acc_bf[:], p_ts[k][:])
                    else:
                        nc.vector.tensor_add(
                            acc_bf[:], acc_bf[:], p_ts[k][:])

                for k in range(nkb):
                    lo, hi = k * PB, (k + 1) * PB
                    p_ts[k] = p_pool.tile(
                        [128, n], bf16, tag="p", name=f"p{k}")
                    s_ts = [s_ps.tile([128, HW], f32, tag="s",
                                      name=f"s{k}_{h}")
                            for h in range(nh)]
                    # left region [0, lo) from (kmT, qpT); right+diag
                    # [lo, n) from (kpT, qmT).  Same-stationary matmuls
                    # back-to-back to amortize LDWEIGHTS.
                    for h in range(nh):
                        c0, c1 = h * HW, (h + 1) * HW
                        for q0 in range(c0, min(c1, lo), 512):
                            e = min(q0 + 512, lo)
                            nc.tensor.matmul(
                                s_ts[h][:, q0 - c0:e - c0], kmT[:, lo:hi],
                                qpT[:, q0:e], start=True, stop=True)
                    for h in range(nh):
                        c0, c1 = h * HW, (h + 1) * HW
                        for q0 in range(c0, c1, 512):
                            q1 = q0 + 512
                            if q1 > lo:
                                b = max(q0, lo)
                                nc.tensor.matmul(
                                    s_ts[h][:, b - c0:q1 - c0],
                                    kpT[:, lo:hi],
                                    qmT[:, b:q1], start=True, stop=True)
                        if c0 <= lo < c1:
                            o = lo - c0
                            nc.vector.tensor_mul(
                                s_ts[h][:, o:o + PB], s_ts[h][:, o:o + PB],
                                bands[:, lo:hi])
                        nc.scalar.activation(
                            p_ts[k][:, c0:c1], s_ts[h][:], Act.Exp,
                            bias=lnm[:, k:k + 1])
                    # software pipeline: consume the PREVIOUS strip's p
                    # so the PE never sits behind ctx waiting on this
                    # strip's exps
                    if k > 0:
                        consume(k - 1)
                consume(nkb - 1)

            # ---- epilogue ----------------------------------------------
            # normalize AFTER Wo (per-query scale commutes through the
            # feature-mixing matmul): out = (Wo^T ctx_raw) * bc(1/sums).
            # ctx evacuation (DVE) overlaps the sums ln/exp chain (ACT);
            # banks 4-7 recycle sums -> bc/out.
            lns = const.tile([1, n], f32)
            invr = const.tile([1, n], f32r)
            ctx_bf = const.tile([128, n], bf16)
            bc_sbs = [const.tile([128, 512], f32, name=f"bc_sb{i}")
                      for i in range(2)]
            outT_sb = const.tile([D, n], bf16)

            for c in range(nqc):
                sl = slice(c * 512, (c + 1) * 512)
                nc.vector.tensor_copy(ctx_bf[:, sl], ctxT_ps[:, sl])

            with tc.tile_pool(name="sums_ps", bufs=1, space="PSUM") as sums_pool:
                sums_t = sums_pool.tile([1, n], f32)
                for c in range(nqc):
                    nc.tensor.matmul(
                        sums_t[0:1, c * 512:(c + 1) * 512], ones_bf[:],
                        acc_bf[:, c * 512:(c + 1) * 512],
                        start=True, stop=True)
                for h in range(2):
                    sl = slice(h * 1024, (h + 1) * 1024)
                    nc.scalar.activation(lns[0:1, sl], sums_t[0:1, sl],
                                         Act.Ln)
                    nc.scalar.activation(invr[0:1, sl], lns[0:1, sl],
                                         Act.Exp, scale=-1.0)

            with (
                tc.tile_pool(name="bc_ps", bufs=2, space="PSUM") as bc_pool,
                tc.tile_pool(name="o_ps", bufs=2, space="PSUM") as o_pool,
            ):
                for c in range(nqc):
                    sl = slice(c * 512, (c + 1) * 512)
                    bc_ps = bc_pool.tile([128, 512], f32, tag="bc")
                    nc.tensor.matmul(
                        bc_ps, ones_r[:], invr[0:1, sl],
                        start=True, stop=True)
                    bc_s = bc_sbs[c % 2]
                    nc.scalar.activation(bc_s[:], bc_ps, Act.Copy)
                    o_t = o_pool.tile([D, 512], f32, tag="o")
                    nc.tensor.matmul(
                        o_t, wo[:], ctx_bf[:, sl], start=True, stop=True)
                    nc.vector.tensor_mul(outT_sb[:, sl], o_t, bc_s[:])
                    eng = nc.sync if c % 2 == 0 else nc.scalar
                    eng.dma_start(outT_d[:, sl], outT_sb[:, sl])

    orig_to_json = nc.to_json_bytes
    nc.to_json_bytes = lambda *a, **kw: _split_drain_waits(orig_to_json(*a, **kw))
    return nc


def _in_maps(inputs, allele_sizes, mask, Wq, Wk, Wv, Wo):
    import ml_dtypes

    bf = ml_dtypes.bfloat16
    n = inputs.shape[1]
    nkb = n // PB
    wq = np.ascontiguousarray(Wq / np.sqrt(np.float32(D))).astype(bf)
    wk = np.ascontiguousarray(Wk).astype(bf)
    wv = np.ascontiguousarray(Wv).astype(bf)
    wo = np.ascontiguousarray(Wo).astype(bf)
    maps = []
    perms = []
    for b in range(inputs.shape[0]):
        a_raw = np.asarray(allele_sizes[b], dtype=np.float64)
        perm = np.argsort(a_raw, kind="stable")
        perms.append(perm)
        a = a_raw[perm]
        a = a - a.mean()          # shift-invariant; shrinks e^{±la} range
        x = np.asarray(inputs[b], dtype=np.float64)[perm]
        m = np.asarray(mask[b], dtype=np.float32)[perm]
        em = np.exp(-LAMBDA_DECAY * a)
        ep = np.exp(LAMBDA_DECAY * a)
        xm = (x * em[:, None]).astype(bf)
        xp = (x * ep[:, None]).astype(bf)
        xb = x.astype(bf)
        # diagonal fix-up blocks: bands[p, 128k+c] =
        # exp(2*lambda*min(a[128k+c] - a[128k+p], 0)), [128, n] fp16
        ab = a.reshape(nkb, PB)
        bands = np.exp(
            2.0 * LAMBDA_DECAY
            * np.minimum(ab[:, None, :] - ab[:, :, None], 0.0)
        )  # [nkb, PB(p), PB(c)]
        bands = np.ascontiguousarray(
            bands.transpose(1, 0, 2).reshape(PB, n)).astype(np.float16)
        maps.append({
            "xT": np.ascontiguousarray(xb.T),
            "xmT": np.ascontiguousarray(xm.T),
            "xpT": np.ascontiguousarray(xp.T),
            "bands": bands,
            "lnm": np.ascontiguousarray(
                np.log(m.reshape(nkb, PB).T,
                       where=m.reshape(nkb, PB).T > 0,
                       out=np.full((PB, nkb), -np.inf, dtype=np.float32))),
            "wq": wq, "wk": wk, "wv": wv, "wo": wo,
        })
    return maps, perms


LAST_RESULTS = None


def kernel(inputs, allele_sizes, mask, Wq, Wk, Wv, Wo, **run_kwargs):
    global LAST_RESULTS
    from concourse.bass_utils import run_bass_kernel_spmd

    key = ("nc", inputs.shape[1])
    if key not in _CACHE:
        _CACHE[key] = _build(n=inputs.shape[1])
    nc = _CACHE[key]
    maps, perms = _in_maps(inputs, allele_sizes, mask, Wq, Wk, Wv, Wo)
    res = run_bass_kernel_spmd(nc, maps, list(range(len(maps))), **run_kwargs)
    LAST_RESULTS = res
    outs = []
    for b, perm in enumerate(perms):
        o_sorted = np.asarray(
            res.results[b]["outT"].T, dtype=np.float32)  # [n, D], sorted
        o = np.empty_like(o_sorted)
        o[perm] = o_sorted
        outs.append(o)
    return np.stack(outs).astype(np.float32)


# revision 31
# speedup vs baseline: 1.3687x; 1.0036x over previous
"""Distance-weighted self-attention on 8 Trainium2 NeuronCores.

Data-parallel over batch: B=8 batches -> 1 batch element per core, no
collectives.  Per core (N=2048 tokens, D=128):

  q = x Wq / sqrt(D), k = x Wk, v = x Wv
  l[i,j] = (q_i . k_j) * exp(-lambda |a_i - a_j|)
  out = softmax_j(l) V Wo

Tokens are SORTED by allele size on the host (attention is
permutation-equivariant).  After sorting, for a key strip covering
sorted positions [128k, 128k+128) the decay factorizes around the
diagonal:
  exp(-l|a_m - a_p|) = (e^{-l a_m} e^{+l a_p})   for a_m >= a_p
                     = (e^{+l a_m} e^{-l a_p})   for a_m <= a_p
The host pre-scales x into xm = x*e^{-l a} and xp = x*e^{+l a}; the
decayed scores come straight out of the Q/K matmuls.  Only the 16
diagonal 128x128 blocks need a fix-up multiply by
exp(2*lambda*min(a_m - a_p, 0)).

Pipeline (all matmul inputs bf16; PSUM accumulation fp32):
  - chunked DMA loads so the first score matmul runs ~3us in (keeps
    the PE HAM clock ramped; dummy warmup matmuls cover the start)
  - projections qm/qp/km/kp/v: bf16 matmuls, DVE evacuates PSUM->bf16
  - per key strip: scores into [128,1024] PSUM tiles (x2 buffers,
    2 banks each; ctx holds the other 4 banks), diagonal fix-up on
    DVE, ONE wide exp per tile on ACT writing p as bf16 (bias carries
    ln(mask))
  - ctx^T accumulates over strips in PSUM via bf16 matmuls
  - softmax sums: DVE running bf16 accumulator over p (keeps the
    ~14us of ones-matmuls off the busy tensor engine), finished by
    four ones-matmuls into PSUM banks 4-7 after the last strip
  - epilogue per 512-query chunk: 1/sums = exp(-ln(sums)), PE
    broadcast to 128 partitions, DVE scales ctx, Wo matmul, DMA out
    (chunks pipeline; banks 4-7 recycled sums->bc/out)
"""

import numpy as np

B, N, D = 8, 2048, 128
PB = 128            # keys per strip (partition block)
LAMBDA_DECAY = 0.1

_CACHE = {}


def _split_drain_waits(bir: bytes, limit: int = 1) -> bytes:
    """This container's walrus rejects instructions carrying more than
    `limit` sync waits ("Too many sync wait commands", setupSyncWait).
    Tile freely attaches several waits to one instruction.  For any
    over-limit instruction, hoist the overflow waits onto same-engine
    EventSemaphore instructions inserted immediately before it
    (same-engine program order preserves the semantics)."""
    import json

    m = json.loads(bir)

    def fix(obj):
        if isinstance(obj, dict):
            if "instructions" in obj and isinstance(obj["instructions"], list):
                out = []
                for ins in obj["instructions"]:
                    si = ins.get("sync_info")
                    if si and si.get("on_wait") and len(si["on_wait"]) > limit:
                        waits = si["on_wait"]
                        chunks = [
                            waits[i:i + limit]
                            for i in range(0, len(waits), limit)
                        ]
                        for j, ch in enumerate(chunks[:-1]):
                            out.append({
                                "name": f"{ins['name']}_w{j}",
                                "opcode": "EventSemaphore",
                                "engine": ins["engine"],
                                "debug": ins.get("debug", 0),
                                "ins": [],
                                "outs": [],
                                "sync_info": {"on_update": [], "on_wait": ch},
                            })
                        si["on_wait"] = chunks[-1]
                    out.append(ins)
                obj["instructions"] = out
            for v in obj.values():
                fix(v)
        elif isinstance(obj, list):
            for v in obj:
                fix(v)

    fix(m)
    return json.dumps(m).encode()


def _build(n=N):
    from contextlib import ExitStack

    import concourse.bass as bass
    import concourse.tile as tile
    from concourse import mybir

    f32 = mybir.dt.float32
    f32r = mybir.dt.float32r
    bf16 = mybir.dt.bfloat16
    Act = mybir.ActivationFunctionType
    Alu = mybir.AluOpType

    nkb = n // PB          # 16 key strips
    nqc = n // 512         # 4 query chunks
    HW = 1024              # exp tile width (2 PSUM banks)
    nh = n // HW           # halves per strip

    f16 = mybir.dt.float16

    nc = bass.Bass("TRN2", target_bir_lowering=False, debug=False)
    xT_d = nc.declare_dram_parameter("xT", [D, n], bf16, isOutput=False)
    xmT_d = nc.declare_dram_parameter("xmT", [D, n], bf16, isOutput=False)
    xpT_d = nc.declare_dram_parameter("xpT", [D, n], bf16, isOutput=False)
    bands_d = nc.declare_dram_parameter("bands", [128, n], f16, isOutput=False)
    lnm_d = nc.declare_dram_parameter("lnm", [128, nkb], f32, isOutput=False)
    wq_d = nc.declare_dram_parameter("wq", [D, D], bf16, isOutput=False)
    wk_d = nc.declare_dram_parameter("wk", [D, D], bf16, isOutput=False)
    wv_d = nc.declare_dram_parameter("wv", [D, D], bf16, isOutput=False)
    wo_d = nc.declare_dram_parameter("wo", [D, D], bf16, isOutput=False)
    outT_d = nc.declare_dram_parameter("outT", [D, n], bf16, isOutput=True)

    with tile.TileContext(nc) as tc:
        with ExitStack() as ctx:
            const = ctx.enter_context(tc.tile_pool(name="const", bufs=1))

            # ---- loads: one ring per engine, earliest-needed first -----
            # sync/scalar are HWDGE rings, gpsimd software ring; spread
            # the big tensors and order by first use
            wq = const.tile([D, D], bf16, tag="wq")
            wk = const.tile([D, D], bf16, tag="wk")
            wv = const.tile([D, D], bf16, tag="wv")
            wo = const.tile([D, D], bf16, tag="wo")
            lnm = const.tile([128, nkb], f32, tag="lnm")
            bands = const.tile([128, n], f16, tag="bands")
            xT = const.tile([D, n], bf16, tag="xT")
            xmT = const.tile([D, n], bf16, tag="xmT")
            xpT = const.tile([D, n], bf16, tag="xpT")

            nc.gpsimd.dma_start(bands[:, 0:256], bands_d[:, 0:256])
            nc.gpsimd.dma_start(wq[:], wq_d[:])
            nc.gpsimd.dma_start(wk[:], wk_d[:])
            nc.gpsimd.dma_start(lnm[:], lnm_d[:])
            h2 = n // 2
            for c in range(2):
                sl = slice(c * h2, (c + 1) * h2)
                nc.sync.dma_start(xmT[:, sl], xmT_d[:, sl])
                nc.scalar.dma_start(xpT[:, sl], xpT_d[:, sl])
                nc.gpsimd.dma_start(xT[:, sl], xT_d[:, sl])
            nc.gpsimd.dma_start(wv[:], wv_d[:])
            nc.gpsimd.dma_start(wo[:], wo_d[:])
            # band blocks for later strips ride the HWDGE ring tails
            nc.sync.dma_start(bands[:, 256:1152], bands_d[:, 256:1152])
            nc.scalar.dma_start(bands[:, 1152:n], bands_d[:, 1152:n])

            ones_bf = const.tile([128, 1], bf16)
            nc.vector.memset(ones_bf[:], 1.0)
            ones_f = const.tile([1, 128], f32)
            nc.vector.memset(ones_f[:], 1.0)
            ones_r = const.tile([1, 128], f32r)
            nc.vector.tensor_copy(ones_r[:], ones_f[:])

            # ---- projections (bf16), PE warmup -------------------------
            qmT = const.tile([D, n], bf16)   # q from xm  (right region)
            qpT = const.tile([D, n], bf16)   # q from xp  (left region)
            kmT = const.tile([D, n], bf16)   # k from xm  (left stationary)
            kpT = const.tile([D, n], bf16)   # k from xp  (right stationary)
            v_sb = const.tile([128, n], bf16)  # strip k at cols [128k,...)

            # PE HAM warmup: dummy matmuls with no DMA deps flip the PE
            # clock gate to 8/8 during the load window
            warm_w = const.tile([128, 128], f32)
            warm_x = const.tile([128, 512], f32)
            nc.vector.memset(warm_w[:], 0.5)
            nc.vector.memset(warm_x[:], 0.5)


            # projections, need-ordered: strip 0 scores need kp strip 0
            # and ALL qm chunks; km/qp follow for later strips; v last.
            # Evacuations split ACT (first few, unblocks strip 0) / DVE.
            with tc.tile_pool(name="proj_ps", bufs=3, space="PSUM") as proj_ps:
                wt = proj_ps.tile([128, 512], f32, tag="warm")
                for i in range(2):
                    nc.tensor.matmul(
                        wt, warm_w[:], warm_x[:],
                        start=(i == 0), stop=(i == 1))

                def v_group(k4):
                    t = proj_ps.tile([128, 4 * PB], f32, tag="proj",
                                     name=f"vps{k4}")
                    for k in range(k4, k4 + 4):
                        nc.tensor.matmul(
                            t[:, (k - k4) * PB:(k - k4 + 1) * PB],
                            xT[:, k * PB:(k + 1) * PB], wv[:],
                            start=True, stop=True)
                    nc.vector.tensor_copy(
                        v_sb[:, k4 * PB:(k4 + 4) * PB], t)

                # strictly need-ordered: strip 0 needs kp-c0 + qm-all;
                # strip 1 adds km-c0/qp-c0; consume(0) needs v0; later
                # chunks arrive just-in-time.  First three evacuations
                # ride ACT (idle until the first exp), rest DVE.
                jobs = [(kpT, wk, xpT, 0), (qmT, wq, xmT, 0),
                        (qmT, wq, xmT, 1), (qmT, wq, xmT, 2),
                        (qmT, wq, xmT, 3), (kmT, wk, xmT, 0),
                        (qpT, wq, xpT, 0), "v0",
                        (kpT, wk, xpT, 1), (kmT, wk, xmT, 1),
                        (qpT, wq, xpT, 1), "v4",
                        (kpT, wk, xpT, 2), (kmT, wk, xmT, 2),
                        (qpT, wq, xpT, 2), "v8",
                        (kpT, wk, xpT, 3), (kmT, wk, xmT, 3),
                        (qpT, wq, xpT, 3), "v12"]
                for j, job in enumerate(jobs):
                    if isinstance(job, str):
                        v_group(int(job[1:]) * 1)
                        continue
                    dst, w, src, c = job
                    sl = slice(c * 512, (c + 1) * 512)
                    t = proj_ps.tile([D, 512], f32, tag="proj")
                    nc.tensor.matmul(
                        t, w[:], src[:, sl], start=True, stop=True)
                    if j < 3:
                        nc.scalar.activation(dst[:, sl], t, Act.Copy)
                    else:
                        nc.vector.tensor_copy(dst[:, sl], t)

            # ---- main loop over key strips ------------------------------
            acc_ps = ctx.enter_context(
                tc.tile_pool(name="acc_ps", bufs=1, space="PSUM"))
            ctxT_ps = acc_ps.tile([128, n], f32)

            acc_bf = const.tile([128, n], bf16)   # running softmax sums

            with (
                tc.tile_pool(name="s_ps", bufs=2, space="PSUM") as s_ps,
                tc.tile_pool(name="p_sb", bufs=3) as p_pool,
            ):
                p_ts = [None] * nkb

                def consume(k):
                    # ctx^T += v_strip^T @ p_k, and softmax-sum acc
                    lo, hi = k * PB, (k + 1) * PB
                    for c in range(nqc):
                        nc.tensor.matmul(
                            ctxT_ps[:, c * 512:(c + 1) * 512],
                            v_sb[:, lo:hi],
                            p_ts[k][:, c * 512:(c + 1) * 512],
                            start=(k == 0), stop=(k == nkb - 1))
                    if k == 0:
                        nc.vector.tensor_copy(acc_bf[:], p_ts[k][:])
                    else:
                        nc.vector.tensor_add(
                            acc_bf[:], acc_bf[:], p_ts[k][:])

                for k in range(nkb):
                    lo, hi = k * PB, (k + 1) * PB
                    p_ts[k] = p_pool.tile(
                        [128, n], bf16, tag="p", name=f"p{k}")
                    s_ts = [s_ps.tile([128, HW], f32, tag="s",
                                      name=f"s{k}_{h}")
                            for h in range(nh)]
                    # left region [0, lo) from (kmT, qpT); right+diag
                    # [lo, n) from (kpT, qmT).  Same-stationary matmuls
                    # back-to-back to amortize LDWEIGHTS.
                    for h in range(nh):
                        c0, c1 = h * HW, (h + 1) * HW
                        for q0 in range(c0, min(c1, lo), 512):
                            e = min(q0 + 512, lo)
                            nc.tensor.matmul(
                                s_ts[h][:, q0 - c0:e - c0], kmT[:, lo:hi],
                                qpT[:, q0:e], start=True, stop=True)
                    for h in range(nh):
                        c0, c1 = h * HW, (h + 1) * HW
                        for q0 in range(c0, c1, 512):
                            q1 = q0 + 512
                            if q1 > lo:
                                b = max(q0, lo)
                                nc.tensor.matmul(
                                    s_ts[h][:, b - c0:q1 - c0],
                                    kpT[:, lo:hi],
                                    qmT[:, b:q1], start=True, stop=True)
                        if c0 <= lo < c1:
                            o = lo - c0
                            nc.vector.tensor_mul(
                                s_ts[h][:, o:o + PB], s_ts[h][:, o:o + PB],
                                bands[:, lo:hi])
                        nc.scalar.activation(
                            p_ts[k][:, c0:c1], s_ts[h][:], Act.Exp,
                            bias=lnm[:, k:k + 1])
                    # software pipeline: consume the PREVIOUS strip's p
                    # so the PE never sits behind ctx waiting on this
                    # strip's exps
                    if k > 0:
                        consume(k - 1)
                consume(nkb - 1)

            # ---- epilogue ----------------------------------------------
            # normalize AFTER Wo (per-query scale commutes through the
            # feature-mixing matmul): out = (Wo^T ctx_raw) * bc(1/sums).
            # ctx evacuation (DVE) overlaps the sums ln/exp chain (ACT);
            # banks 4-7 recycle sums -> bc/out.
            lns = const.tile([1, n], f32)
            invr = const.tile([1, n], f32r)
            ctx_bf = const.tile([128, n], bf16)
            bc_sbs = [const.tile([128, 512], f32, name=f"bc_sb{i}")
                      for i in range(2)]
            outT_sb = const.tile([D, n], bf16)

            for c in range(nqc):
                sl = slice(c * 512, (c + 1) * 512)
                nc.vector.tensor_copy(ctx_bf[:, sl], ctxT_ps[:, sl])

            with tc.tile_pool(name="sums_ps", bufs=1, space="PSUM") as sums_pool:
                sums_t = sums_pool.tile([1, n], f32)
                for c in range(nqc):
                    nc.tensor.matmul(
                        sums_t[0:1, c * 512:(c + 1) * 512], ones_bf[:],
                        acc_bf[:, c * 512:(c + 1) * 512],
                        start=True, stop=True)
                for h in range(2):
                    sl = slice(h * 1024, (h + 1) * 1024)
                    nc.scalar.activation(lns[0:1, sl], sums_t[0:1, sl],
                                         Act.Ln)
                    nc.scalar.activation(invr[0:1, sl], lns[0:1, sl],
                                         Act.Exp, scale=-1.0)

            with (
                tc.tile_pool(name="bc_ps", bufs=2, space="PSUM") as bc_pool,
                tc.tile_pool(name="o_ps", bufs=2, space="PSUM") as o_pool,
            ):
                for c in range(nqc):
                    sl = slice(c * 512, (c + 1) * 512)
                    bc_ps = bc_pool.tile([128, 512], f32, tag="bc")
                    nc.tensor.matmul(
                        bc_ps, ones_r[:], invr[0:1, sl],
                        start=True, stop=True)
                    bc_s = bc_sbs[c % 2]
                    nc.scalar.activation(bc_s[:], bc_ps, Act.Copy)
                    o_t = o_pool.tile([D, 512], f32, tag="o")
                    nc.tensor.matmul(
                        o_t, wo[:], ctx_bf[:, sl], start=True, stop=True)
                    nc.vector.tensor_mul(outT_sb[:, sl], o_t, bc_s[:])
                    eng = nc.sync if c % 2 == 0 else nc.scalar
                    eng.dma_start(outT_d[:, sl], outT_sb[:, sl])

    orig_to_json = nc.to_json_bytes
    nc.to_json_bytes = lambda *a, **kw: _split_drain_waits(orig_to_json(*a, **kw))
    return nc


def _in_maps(inputs, allele_sizes, mask, Wq, Wk, Wv, Wo):
    import ml_dtypes

    bf = ml_dtypes.bfloat16
    n = inputs.shape[1]
    nkb = n // PB
    wq = np.ascontiguousarray(Wq / np.sqrt(np.float32(D))).astype(bf)
    wk = np.ascontiguousarray(Wk).astype(bf)
    wv = np.ascontiguousarray(Wv).astype(bf)
    wo = np.ascontiguousarray(Wo).astype(bf)
    maps = []
    perms = []
    for b in range(inputs.shape[0]):
        a_raw = np.asarray(allele_sizes[b], dtype=np.float64)
        perm = np.argsort(a_raw, kind="stable")
        perms.append(perm)
        a = a_raw[perm]
        a = a - a.mean()          # shift-invariant; shrinks e^{±la} range
        x = np.asarray(inputs[b], dtype=np.float64)[perm]
        m = np.asarray(mask[b], dtype=np.float32)[perm]
        em = np.exp(-LAMBDA_DECAY * a)
        ep = np.exp(LAMBDA_DECAY * a)
        xm = (x * em[:, None]).astype(bf)
        xp = (x * ep[:, None]).astype(bf)
        xb = x.astype(bf)
        # diagonal fix-up blocks: bands[p, 128k+c] =
        # exp(2*lambda*min(a[128k+c] - a[128k+p], 0)), [128, n] fp16
        ab = a.reshape(nkb, PB)
        bands = np.exp(
            2.0 * LAMBDA_DECAY
            * np.minimum(ab[:, None, :] - ab[:, :, None], 0.0)
        )  # [nkb, PB(p), PB(c)]
        bands = np.ascontiguousarray(
            bands.transpose(1, 0, 2).reshape(PB, n)).astype(np.float16)
        maps.append({
            "xT": np.ascontiguousarray(xb.T),
            "xmT": np.ascontiguousarray(xm.T),
            "xpT": np.ascontiguousarray(xp.T),
            "bands": bands,
            "lnm": np.ascontiguousarray(
                np.log(m.reshape(nkb, PB).T,
                       where=m.reshape(nkb, PB).T > 0,
                       out=np.full((PB, nkb), -np.inf, dtype=np.float32))),
            "wq": wq, "wk": wk, "wv": wv, "wo": wo,
        })
    return maps, perms


LAST_RESULTS = None


def kernel(inputs, allele_sizes, mask, Wq, Wk, Wv, Wo, **run_kwargs):
    global LAST_RESULTS
    from concourse.bass_utils import run_bass_kernel_spmd

    key = ("nc", inputs.shape[1])
    if key not in _CACHE:
        _CACHE[key] = _build(n=inputs.shape[1])
    nc = _CACHE[key]
    maps, perms = _in_maps(inputs, allele_sizes, mask, Wq, Wk, Wv, Wo)
    res = run_bass_kernel_spmd(nc, maps, list(range(len(maps))), **run_kwargs)
    LAST_RESULTS = res
    outs = []
    for b, perm in enumerate(perms):
        o_sorted = np.asarray(
            res.results[b]["outT"].T, dtype=np.float32)  # [n, D], sorted
        o = np.empty_like(o_sorted)
        o[perm] = o_sorted
        outs.append(o)
    return np.stack(outs).astype(np.float32)
